# revision 23
# baseline (speedup 1.0000x reference)
"""Trainium2 Bass kernel for a dense transformer block (LN->causal attn->res->LN->MLP->res).

Shapes (hardcoded): x [2, 2048, 1024], 16 heads, head_dim 64, MLP hidden 4096, fp32 in/out.

Sharding: 8 cores = (batch b in {0,1}) x (sequence chunk j in {0..3}, 512 tokens).
Each core receives its batch's full 2048-token context, ROTATED so that its own
chunk sits in the last 512 positions.  This makes the SPMD program identical on
every core: causality is enforced by per-core DATA only —
  * a per-key additive bias (0 for past chunks, -30000 for future chunks) folded
    into the softmax exp on the ACT engine, and
  * a fixed triangular 0/1 mask multiplied onto the last 4 key tiles (the
    diagonal block, which is always the last 512 keys after rotation).
The core computes LN1 + K/V over the whole context (data-parallel replication),
Q/attention/LN2/MLP only for its own 512 tokens, and writes its [512, 1024]
slice of the output.  No cross-core communication.

Numerics: MLP weights and all PE accumulation are fp32-class; matmul operands
use float32r (full-rate reduced-precision fp32 multiply) except the QKV
*production* inputs (xn^T and w_qkv/w_v) and attention probabilities/V, which
are bf16 to fit SBUF — scores themselves are computed f32r from f32r Q^T/K^T.

Attention is key-tile-outer in 4 rounds of 4 heads: for each key tile,
S^T = K^T.T @ Q^T for two head-pairs (row-packed 64+64 matmuls into a 2-bank
PSUM tile), one paired exp on ACT (per-key causal bias via the activation bias
operand), then immediate AV accumulation into per-head [65, 512] PSUM — the
65th V column is all-ones and accumulates the softmax denominator.  This lets
exp for round 0 overlap V production on the PE.  K^T bounces through DRAM;
V stays resident in SBUF.
"""

from contextlib import ExitStack

import numpy as np

import concourse.bacc as bacc
import concourse.mybir as mybir
import concourse.tile as tile
from concourse.masks import make_identity

F32 = mybir.dt.float32
F32R = mybir.dt.float32r
BF16 = mybir.dt.bfloat16
AF = mybir.ActivationFunctionType
ALU = mybir.AluOpType

B = 2
T = 2048
D = 1024
H = 16
HD = 64
HDA = HD + 1  # +1 denominator column per head
MLP = 4096
NQ = 512  # tokens per core
CTX = T
EPS = 1e-5
NEG = -30000.0

N_CORES = 8
P = 128

KT_T = CTX // P  # 16 key tiles
D_T = D // P  # 8
Q_T = NQ // P  # 4
M_T = MLP // P  # 32
VA = H * HDA  # 1040 augmented V width


def build_program(loop_n: int = 1, bv_nonzero: bool = False):
    """Emit the SPMD Bass program. Returns finalized nc."""
    nc = bacc.Bacc("TRN2", target_bir_lowering=False)

    xc = nc.dram_tensor("xc", [CTX, D], F32, kind="ExternalInput")
    wqk = nc.dram_tensor("wqk", [D, 2 * D], BF16, kind="ExternalInput")
    bqk = nc.dram_tensor("bqk", [P, 2 * D_T], F32, kind="ExternalInput")
    wva = nc.dram_tensor("wva", [D, VA], BF16, kind="ExternalInput")
    bva = nc.dram_tensor("bva", [1, VA], BF16, kind="ExternalInput")
    biask = nc.dram_tensor("biask", [P, KT_T], F32, kind="ExternalInput")
    trimask = nc.dram_tensor("trimask", [P, 4 * 2 * NQ], BF16, kind="ExternalInput")
    wfc = nc.dram_tensor("wfc", [D, MLP], F32R, kind="ExternalInput")
    bfc = nc.dram_tensor("bfc", [P, M_T], F32, kind="ExternalInput")
    wproj = nc.dram_tensor("wproj", [MLP, D], F32R, kind="ExternalInput")
    bproj = nc.dram_tensor("bproj", [P, D_T], F32, kind="ExternalInput")
    out = nc.dram_tensor("out", [NQ, D], F32, kind="ExternalOutput")

    with tile.TileContext(nc) as tc:
        with ExitStack() as ctx:
            if loop_n > 1:
                ctx.enter_context(tc.For_i(0, loop_n, 1))
            const = ctx.enter_context(tc.tile_pool(name="const", bufs=1))
            identity = const.tile([P, P], F32)
            make_identity(nc, identity)
            ones1 = const.tile([1, P], BF16)
            nc.vector.memset(ones1, 1.0)
            eps_t = const.tile([P, 1], F32)
            nc.vector.memset(eps_t, EPS)
            bqk_sb = const.tile([P, 2 * D_T], F32)
            nc.sync.dma_start(bqk_sb, bqk[:, :])
            bva_sb = const.tile([1, VA], BF16)
            nc.sync.dma_start(bva_sb, bva[:, :])
            biask_sb = const.tile([P, KT_T], F32)
            nc.sync.dma_start(biask_sb, biask[:, :])

            drampool = ctx.enter_context(
                tc.tile_pool(name="drampool", bufs=1, space="DRAM")
            )
            ktdram = drampool.tile([D, CTX], F32R)

            # Long-lived LEFT pools (closed in reverse order at end of P3)
            xnt_cm = tc.tile_pool(name="xnt", bufs=1)
            xnt_pool = xnt_cm.__enter__()
            xnT = [xnt_pool.tile([P, CTX], BF16, name=f"xnT{i}") for i in range(D_T)]
            qt_cm = tc.tile_pool(name="qt", bufs=1)
            qt_pool = qt_cm.__enter__()
            QT = [qt_pool.tile([P, NQ], F32R, name=f"QT{i}") for i in range(D_T)]
            vsb_cm = tc.tile_pool(name="vsb", bufs=1)
            vsb_pool = vsb_cm.__enter__()
            VSB = [vsb_pool.tile([P, VA], BF16, name=f"VSB{i}") for i in range(KT_T)]

            # RIGHT pools (live into P4/P5)
            yt_pool = ctx.enter_context(tc.tile_pool(name="yt", bufs=1, side="right"))
            YT = [yt_pool.tile([P, NQ], F32, name=f"YT{i}") for i in range(D_T)]
            x2_pool = ctx.enter_context(tc.tile_pool(name="x2", bufs=1, side="right"))
            X2 = [x2_pool.tile([P, D], F32, name=f"X2{i}") for i in range(Q_T)]
            l2t_pool = ctx.enter_context(
                tc.tile_pool(name="l2t", bufs=1, side="right")
            )
            L2T = [l2t_pool.tile([P, NQ], F32R, name=f"L2T{i}") for i in range(D_T)]

            # ---------------- P1: load x, LN1, transpose -> xnT (bf16) ---------
            with tc.tile_pool(name="p1work", bufs=4) as p1w, tc.tile_pool(
                name="p1stat", bufs=4
            ) as p1s, tc.tile_pool(name="p1ps", bufs=6, space="PSUM") as p1ps:
                for tt in range(KT_T):
                    xt = p1w.tile([P, D], F32, tag="xt")
                    nc.sync.dma_start(xt, xc[tt * P : (tt + 1) * P, :])
                    stats = p1s.tile([P, 2, 6], F32, tag="stats")
                    for g in range(2):
                        nc.vector.bn_stats(
                            stats[:, g, :], xt[:, g * 512 : (g + 1) * 512]
                        )
                    mv = p1s.tile([P, 2], F32, tag="mv")
                    nc.vector.bn_aggr(mv, stats)
                    sd = p1s.tile([P, 1], F32, tag="sd")
                    nc.scalar.activation(sd, mv[:, 1:2], AF.Sqrt, bias=eps_t)
                    rstd = p1s.tile([P, 1], F32, tag="rstd")
                    nc.vector.reciprocal(rstd, sd)
                    nmb = p1s.tile([P, 1], F32, tag="nmb")
                    nc.vector.tensor_scalar(
                        nmb, mv[:, 0:1], rstd, -1.0, ALU.mult, ALU.mult
                    )
                    xn = p1w.tile([P, D], F32, tag="xn")
                    nc.scalar.activation(xn, xt, AF.Identity, bias=nmb, scale=rstd)
                    for dt_ in range(D_T):
                        tp = p1ps.tile([P, P], F32, tag="tp")
                        nc.tensor.transpose(
                            tp, xn[:, dt_ * P : (dt_ + 1) * P], identity
                        )
                        if dt_ % 2 == 0:
                            nc.vector.tensor_copy(
                                xnT[dt_][:, tt * P : (tt + 1) * P], tp
                            )
                        else:
                            nc.scalar.copy(xnT[dt_][:, tt * P : (tt + 1) * P], tp)

            # ---------------- P2a: Q^T ----------------
            with tc.tile_pool(name="p2q", bufs=2) as p2q, tc.tile_pool(
                name="p2qps", bufs=3, space="PSUM"
            ) as p2qps:
                for mt in range(D_T):
                    ws = p2q.tile([P, D_T, P], BF16, tag="wsq")
                    nc.sync.dma_start(
                        ws,
                        wqk[:, mt * P : (mt + 1) * P].rearrange(
                            "(a p) c -> p a c", p=P
                        ),
                    )
                    ps = p2qps.tile([P, NQ], F32, tag="ps")
                    for kt_ in range(D_T):
                        nc.tensor.matmul(
                            ps,
                            ws[:, kt_, :],
                            xnT[kt_][:, CTX - NQ :],
                            start=(kt_ == 0),
                            stop=(kt_ == D_T - 1),
                        )
                    nc.scalar.activation(
                        QT[mt], ps, AF.Identity, bias=bqk_sb[:, mt : mt + 1]
                    )

            # ---------------- P2b: K^T -> ktdram (round-0 pairs first) -------
            with tc.tile_pool(name="p2k", bufs=2) as p2k, tc.tile_pool(
                name="p2kps", bufs=3, space="PSUM"
            ) as p2kps:
                for mt in (0, 1):
                    ws = p2k.tile([P, D_T, P], BF16, tag="wsk")
                    nc.sync.dma_start(
                        ws,
                        wqk[:, D + mt * P : D + (mt + 1) * P].rearrange(
                            "(a p) c -> p a c", p=P
                        ),
                    )
                    for nt in range(CTX // 512):
                        ps = p2kps.tile([P, 512], F32, tag="ps")
                        for kt_ in range(D_T):
                            nc.tensor.matmul(
                                ps,
                                ws[:, kt_, :],
                                xnT[kt_][:, nt * 512 : (nt + 1) * 512],
                                start=(kt_ == 0),
                                stop=(kt_ == D_T - 1),
                            )
                        kev = p2k.tile([P, 512], F32R, tag="kev")
                        nc.vector.tensor_scalar_add(
                            kev, ps, bqk_sb[:, D_T + mt : D_T + mt + 1]
                        )
                        nc.sync.dma_start(
                            ktdram[mt * P : (mt + 1) * P, nt * 512 : (nt + 1) * 512],
                            kev,
                        )

            # ---------------- P2c: V_aug resident ---------------
            # Attention pools open BEFORE V / K2-7 emission so their SBUF/PSUM
            # zones don't land on those pools' freed space (which would add a
            # false wait-for-completion dependency and serialize attention
            # behind all of V/K production).
            p3w_cm = tc.tile_pool(name="p3w", bufs=3)
            p3w = p3w_cm.__enter__()
            p3tri_cm = tc.tile_pool(name="p3tri", bufs=1)
            p3tri = p3tri_cm.__enter__()
            ptp_cm = tc.tile_pool(name="ptp", bufs=4)
            ptp = ptp_cm.__enter__()
            p3s_cm = tc.tile_pool(name="p3s", bufs=2)
            p3s = p3s_cm.__enter__()
            stps_cm = tc.tile_pool(name="stps", bufs=2, space="PSUM")
            stps = stps_cm.__enter__()
            yps_cm = tc.tile_pool(name="yps", bufs=2, space="PSUM")
            yps = yps_cm.__enter__()
            tri_sb = p3tri.tile([P, 4, 2 * NQ], BF16)
            nc.sync.dma_start(
                tri_sb, trimask.rearrange("p (a q) -> p a q", q=2 * NQ)
            )

            wva_cm = tc.tile_pool(name="wvap", bufs=1)
            wva_pool = wva_cm.__enter__()
            wvasb = [wva_pool.tile([P, VA], BF16, name=f"wva{i}") for i in range(D_T)]
            for kt_ in range(D_T):
                nc.sync.dma_start(wvasb[kt_], wva[kt_ * P : (kt_ + 1) * P, :])
            vchunks = [(0, 512), (512, 512), (1024, VA - 1024)]
            with tc.tile_pool(name="p2vps", bufs=2, space="PSUM", side="right") as p2vps:
                for mt in range(KT_T):
                    for c0, cw in vchunks:
                        ps = p2vps.tile([P, 512], F32, tag="ps")
                        for kt_ in range(D_T):
                            nc.tensor.matmul(
                                ps[:, :cw],
                                xnT[kt_][:, mt * P : (mt + 1) * P],
                                wvasb[kt_][:, c0 : c0 + cw],
                                start=(kt_ == 0),
                                stop=(kt_ == D_T - 1 and not bv_nonzero),
                            )
                        if bv_nonzero:
                            nc.tensor.matmul(
                                ps[:, :cw],
                                ones1,
                                bva_sb[:, c0 : c0 + cw],
                                start=False,
                                stop=True,
                            )
                        nc.vector.tensor_copy(VSB[mt][:, c0 : c0 + cw], ps[:, :cw])
                    if not bv_nonzero:
                        ones_cols = VSB[mt].rearrange("p (h c) -> p h c", c=HDA)[
                            :, :, HD : HD + 1
                        ]
                        nc.vector.memset(ones_cols, 1.0)

            # ---------------- P2b': K^T mt 2..7 ----------------
            with tc.tile_pool(name="p2k2", bufs=2) as p2k2, tc.tile_pool(
                name="p2k2ps", bufs=2, space="PSUM", side="right"
            ) as p2k2ps:
                for mt in range(2, D_T):
                    ws = p2k2.tile([P, D_T, P], BF16, tag="wsk2")
                    nc.sync.dma_start(
                        ws,
                        wqk[:, D + mt * P : D + (mt + 1) * P].rearrange(
                            "(a p) c -> p a c", p=P
                        ),
                    )
                    for nt in range(CTX // 512):
                        ps = p2k2ps.tile([P, 512], F32, tag="ps")
                        for kt_ in range(D_T):
                            nc.tensor.matmul(
                                ps,
                                ws[:, kt_, :],
                                xnT[kt_][:, nt * 512 : (nt + 1) * 512],
                                start=(kt_ == 0),
                                stop=(kt_ == D_T - 1),
                            )
                        kev = p2k2.tile([P, 512], F32R, tag="kev2")
                        nc.vector.tensor_scalar_add(
                            kev, ps, bqk_sb[:, D_T + mt : D_T + mt + 1]
                        )
                        nc.sync.dma_start(
                            ktdram[mt * P : (mt + 1) * P, nt * 512 : (nt + 1) * 512],
                            kev,
                        )

            # P3: key-tile-outer attention, 8 rounds x 2 heads
            if True:
                for hp in range(H // 2):
                    ktp = p3w.tile([P, CTX], F32R, name=f"ktp{hp}", tag="ktp")
                    nc.sync.dma_start(ktp, ktdram[hp * P : (hp + 1) * P, :])
                    yp = [
                        yps.tile([HDA, NQ], F32, name=f"yp{hp}_{s}", tag="yp")
                        for s in range(2)
                    ]
                    for kt in range(KT_T):
                        st2 = stps.tile([P, 2 * NQ], F32, tag="st2")
                        for s in range(2):
                            nc.tensor.matmul(
                                st2[:, s * NQ : (s + 1) * NQ],
                                ktp[s * HD : (s + 1) * HD, kt * P : (kt + 1) * P],
                                QT[hp][s * HD : (s + 1) * HD, :],
                                start=True,
                                stop=True,
                                tile_position=(s * HD, 0),
                            )
                        pt2 = ptp.tile([P, 2 * NQ], BF16, tag="pt2")
                        nc.scalar.activation(
                            pt2, st2, AF.Exp, bias=biask_sb[:, kt : kt + 1]
                        )
                        if kt >= KT_T - 4:
                            nc.vector.tensor_mul(
                                pt2, pt2, tri_sb[:, kt - (KT_T - 4), :]
                            )
                        for s in range(2):
                            h = 2 * hp + s
                            nc.tensor.matmul(
                                yp[s],
                                VSB[kt][:, h * HDA : (h + 1) * HDA],
                                pt2[:, s * NQ : (s + 1) * NQ],
                                start=(kt == 0),
                                stop=(kt == KT_T - 1),
                            )
                    for s in range(2):
                        ysb = p3s.tile([HDA, NQ], F32, name=f"ysb{hp}_{s}", tag="ysb")
                        nc.vector.tensor_copy(ysb, yp[s])
                        recip = p3s.tile([1, NQ], F32, tag="recip")
                        nc.vector.reciprocal(recip, ysb[HD : HD + 1, :])
                        rb = p3s.tile([HD, NQ], F32, tag="rb")
                        nc.gpsimd.partition_broadcast(rb, recip)
                        nc.vector.tensor_mul(
                            YT[hp][s * HD : (s + 1) * HD, :], ysb[:HD, :], rb
                        )

            wva_cm.__exit__(None, None, None)
            yps_cm.__exit__(None, None, None)
            stps_cm.__exit__(None, None, None)
            p3s_cm.__exit__(None, None, None)
            ptp_cm.__exit__(None, None, None)
            p3tri_cm.__exit__(None, None, None)
            p3w_cm.__exit__(None, None, None)
            vsb_cm.__exit__(None, None, None)
            qt_cm.__exit__(None, None, None)
            xnt_cm.__exit__(None, None, None)

            # ---------------- P4: residual + LN2 + transpose ----------------
            with tc.tile_pool(name="p4w", bufs=3) as p4w, tc.tile_pool(
                name="p4s", bufs=4
            ) as p4s, tc.tile_pool(name="p4ps", bufs=4, space="PSUM") as p4ps:
                for tt in range(Q_T):
                    xl = p4w.tile([P, D], F32, tag="xl")
                    nc.sync.dma_start(
                        xl, xc[CTX - NQ + tt * P : CTX - NQ + (tt + 1) * P, :]
                    )
                    for mt in range(D_T):
                        tp = p4ps.tile([P, P], F32, tag="tp")
                        nc.tensor.transpose(
                            tp, YT[mt][:, tt * P : (tt + 1) * P], identity
                        )
                        nc.vector.tensor_add(
                            X2[tt][:, mt * P : (mt + 1) * P],
                            xl[:, mt * P : (mt + 1) * P],
                            tp,
                        )
                    stats = p4s.tile([P, 2, 6], F32, tag="stats2")
                    for g in range(2):
                        nc.vector.bn_stats(
                            stats[:, g, :], X2[tt][:, g * 512 : (g + 1) * 512]
                        )
                    mv = p4s.tile([P, 2], F32, tag="mv2")
                    nc.vector.bn_aggr(mv, stats)
                    sd = p4s.tile([P, 1], F32, tag="sd2")
                    nc.scalar.activation(sd, mv[:, 1:2], AF.Sqrt, bias=eps_t)
                    rstd = p4s.tile([P, 1], F32, tag="rstd2")
                    nc.vector.reciprocal(rstd, sd)
                    nmb = p4s.tile([P, 1], F32, tag="nmb2")
                    nc.vector.tensor_scalar(
                        nmb, mv[:, 0:1], rstd, -1.0, ALU.mult, ALU.mult
                    )
                    l2 = p4w.tile([P, D], F32, tag="l2")
                    nc.scalar.activation(l2, X2[tt], AF.Identity, bias=nmb, scale=rstd)
                    for mt in range(D_T):
                        tp = p4ps.tile([P, P], F32, tag="tp")
                        nc.tensor.transpose(tp, l2[:, mt * P : (mt + 1) * P], identity)
                        nc.vector.tensor_copy(L2T[mt][:, tt * P : (tt + 1) * P], tp)

            # ---------------- P5: MLP + final residual ----------------
            with tc.tile_pool(name="h1t", bufs=1) as h1t_pool, tc.tile_pool(
                name="p5w", bufs=2
            ) as p5w, tc.tile_pool(name="p5o", bufs=1) as p5o, tc.tile_pool(
                name="p5ps", bufs=2, space="PSUM"
            ) as p5ps, tc.tile_pool(
                name="p5tps", bufs=4, space="PSUM"
            ) as p5tps:
                bfc_sb = p5o.tile([P, M_T], F32)
                nc.sync.dma_start(bfc_sb, bfc[:, :])
                bproj_sb = p5o.tile([P, D_T], F32)
                nc.sync.dma_start(bproj_sb, bproj[:, :])
                OUT = [p5o.tile([P, D], F32, name=f"OUT{i}") for i in range(Q_T)]
                H1T = [h1t_pool.tile([P, NQ], F32R, name=f"H1T{i}") for i in range(M_T)]
                for mt in range(M_T):
                    ws = p5w.tile([P, D_T, P], F32R, tag="wsf")
                    nc.sync.dma_start(
                        ws,
                        wfc[:, mt * P : (mt + 1) * P].rearrange(
                            "(a p) c -> p a c", p=P
                        ),
                    )
                    ps = p5ps.tile([P, NQ], F32, tag="ps")
                    for kt_ in range(D_T):
                        nc.tensor.matmul(
                            ps,
                            ws[:, kt_, :],
                            L2T[kt_],
                            start=(kt_ == 0),
                            stop=(kt_ == D_T - 1),
                        )
                    nc.vector.tensor_scalar(
                        H1T[mt], ps, bfc_sb[:, mt : mt + 1], 0.0, ALU.add, ALU.max
                    )
                for mt in range(D_T):
                    ws = p5w.tile([P, M_T, P], F32R, tag="wsp")
                    nc.sync.dma_start(
                        ws,
                        wproj[:, mt * P : (mt + 1) * P].rearrange(
                            "(a p) c -> p a c", p=P
                        ),
                    )
                    ps = p5ps.tile([P, NQ], F32, tag="ps")
                    for kt_ in range(M_T):
                        nc.tensor.matmul(
                            ps,
                            ws[:, kt_, :],
                            H1T[kt_],
                            start=(kt_ == 0),
                            stop=(kt_ == M_T - 1),
                        )
                    mlpt = p5w.tile([P, NQ], F32, tag="mlpt")
                    nc.vector.tensor_scalar_add(mlpt, ps, bproj_sb[:, mt : mt + 1])
                    for tt in range(Q_T):
                        tp = p5tps.tile([P, P], F32, tag="tp")
                        nc.tensor.transpose(
                            tp, mlpt[:, tt * P : (tt + 1) * P], identity
                        )
                        nc.vector.tensor_add(
                            OUT[tt][:, mt * P : (mt + 1) * P],
                            X2[tt][:, mt * P : (mt + 1) * P],
                            tp,
                        )
                for tt in range(Q_T):
                    nc.sync.dma_start(out[tt * P : (tt + 1) * P, :], OUT[tt])

    nc.finalize()
    return nc


_PROG = {}


def _get_program(bv_nonzero: bool = False):
    if bv_nonzero not in _PROG:
        _PROG[bv_nonzero] = build_program(bv_nonzero=bv_nonzero)
    return _PROG[bv_nonzero]


def make_in_maps(x, ln1_scale, ln1_shift, w_qkv, b_qkv, ln2_scale, ln2_shift,
                 w_fc, b_fc, w_proj, b_proj):
    """Host-side prep: fold LN affine into weights, prescale Q by 1/sqrt(hd),
    augment V with an all-ones output column per head, build per-core rotated
    context + causal bias/mask data."""
    import ml_dtypes

    bf16 = ml_dtypes.bfloat16

    x = np.asarray(x, np.float32)
    ln1_scale = np.asarray(ln1_scale, np.float32)
    ln1_shift = np.asarray(ln1_shift, np.float32)
    w_qkv = np.asarray(w_qkv, np.float32)
    b_qkv = np.asarray(b_qkv, np.float32)
    ln2_scale = np.asarray(ln2_scale, np.float32)
    ln2_shift = np.asarray(ln2_shift, np.float32)
    w_fc = np.asarray(w_fc, np.float32)
    b_fc = np.asarray(b_fc, np.float32)
    w_proj = np.asarray(w_proj, np.float32)
    b_proj = np.asarray(b_proj, np.float32)

    # fold LN1 affine into qkv weights
    w1 = ln1_scale[:, None] * w_qkv  # [D, 3D]
    b1 = b_qkv + ln1_shift @ w_qkv  # [3D]
    sc = 1.0 / np.sqrt(HD)
    wq = w1[:, :D] * sc
    bq = b1[:D] * sc
    wk = w1[:, D : 2 * D]
    bk = b1[D : 2 * D]
    wv = w1[:, 2 * D :]
    bv = b1[2 * D :]

    wqk_h = np.ascontiguousarray(
        np.concatenate([wq, wk], axis=1).astype(bf16)
    )  # [D, 2D] bf16
    bqk_h = np.ascontiguousarray(
        np.concatenate([bq, bk]).reshape(2 * D_T, P).T
    )  # [128, 16] f32

    wva_h = np.zeros((D, VA), np.float32)
    bva_h = np.zeros((1, VA), np.float32)
    for h in range(H):
        wva_h[:, h * HDA : h * HDA + HD] = wv[:, h * HD : (h + 1) * HD]
        bva_h[0, h * HDA : h * HDA + HD] = bv[h * HD : (h + 1) * HD]
        bva_h[0, h * HDA + HD] = 1.0  # denominator ones column
    wva_h = wva_h.astype(bf16)
    bva_h = bva_h.astype(bf16)

    # fold LN2 affine into fc
    wfc_h = np.ascontiguousarray(ln2_scale[:, None] * w_fc)
    bfc_h = np.ascontiguousarray((b_fc + ln2_shift @ w_fc).reshape(M_T, P).T)
    bproj_h = np.ascontiguousarray(b_proj.reshape(D_T, P).T)  # [128, 8]

    # triangular mask for the diagonal (last) key block, S^T orientation,
    # duplicated for the head-pair layout: [128, 4, 2*NQ]
    kk = np.arange(NQ)[:, None]
    qq = np.arange(NQ)[None, :]
    tri = (kk <= qq).astype(np.float32)  # [512, 512]
    tri4 = tri.reshape(4, P, NQ)
    tri_h = np.ascontiguousarray(
        np.concatenate([tri4, tri4], axis=2).transpose(1, 0, 2).reshape(P, 4 * 2 * NQ)
    ).astype(bf16)

    in_maps = []
    for c in range(N_CORES):
        b, j = divmod(c, 4)
        xb = x[b]  # [T, D]
        xperm = np.roll(xb, -((j + 1) * NQ), axis=0)  # own chunk last
        # after roll, position block p (of 4) holds chunk (j+1+p) % 4
        bias = np.zeros(CTX, np.float32)
        for pblk in range(3):
            cp = (j + 1 + pblk) % 4
            if cp > j:
                bias[pblk * NQ : (pblk + 1) * NQ] = NEG
        biask_h = np.ascontiguousarray(bias.reshape(KT_T, P).T)  # [128, 16]
        in_maps.append(
            {
                "xc": np.ascontiguousarray(xperm),
                "wqk": wqk_h,
                "bqk": bqk_h,
                "wva": wva_h,
                "bva": bva_h,
                "biask": biask_h,
                "trimask": tri_h,
                "wfc": wfc_h,
                "bfc": bfc_h,
                "wproj": np.ascontiguousarray(w_proj),
                "bproj": bproj_h,
            }
        )
    return in_maps


def assemble_output(results):
    out = np.empty((B, T, D), np.float32)
    for c in range(N_CORES):
        b, j = divmod(c, 4)
        out[b, j * NQ : (j + 1) * NQ, :] = results[c]["out"]
    return out


def kernel(**inputs) -> np.ndarray:
    from concourse.bass_utils import run_bass_kernel_spmd

    in_maps = make_in_maps(**inputs)
    bva = np.asarray(in_maps[0]["bva"], np.float32)[0]
    mask = np.ones(VA, bool)
    mask[HD::HDA] = False  # the ones columns
    nc = _get_program(bv_nonzero=bool(np.any(bva[mask] != 0.0)))
    res = run_bass_kernel_spmd(nc, in_maps, core_ids=list(range(N_CORES)))
    return assemble_output(res.results)
